# revision 11
# baseline (speedup 1.0000x reference)
"""Trainium2 Bass kernel for nn_DEQSolver_2894807957574.

Math: the reference runs 40 Anderson-accelerated fixed-point iterations of the
ISTA map  f(z) = softshrink((1-rho)*z + rho*x0, rho*lam)  and then applies one
more ISTA step.  The map is a contraction with factor |1-rho| (= 0.1 here), so
in fp32 the iterate fully converges to the unique fixed point
z* = softshrink(x0, lam), and the final ISTA step maps the fixed point to
itself.  The returned value therefore has the closed form

    c1 = clamp(x0, +-lam)
    u  = x0 + (-(1-rho))*c1          # = (1-rho)*z* + rho*x0
    out = u - clamp(u, +-rho*lam)    # = softshrink(u, rho*lam)

computed with exactly this fp32 op order; that was verified BITWISE identical
(absmax diff 0.0) to the full jax reference on the target inputs.

Sharding: pure data parallel - batch dim 8, one sample per NeuronCore.  Each
core streams its 3 MB sample HBM->SBUF in chunks, applies the elementwise ops
(optionally split across DVE / ACT / GPSIMD so the ~358 GB/s HBM path stays
the bottleneck), and streams the 3 MB result back.
"""

import numpy as np

import concourse.bass as bass
import concourse.mybir as mybir
from concourse.bass_utils import run_bass_kernel_spmd
from concourse.tile import TileContext

_B, _C, _H, _W = 8, 3, 512, 512
_P = 128                      # SBUF partitions
_FD = (_C * _H * _W) // _P    # 6144 free-dim elements per partition
_NCORES = 8
_NCHUNK = 4                   # chunks along the free dim (768 KB per DMA)
_VARIANT = "b"

_f32 = mybir.dt.float32

# variant -> (m_engine, soft_mode, sub_engine)
#   m_engine: engine computing m = c1 * (-(1-rho))
#   soft_mode: "relu"  -> r3=relu(u-t), r4=relu(-u-t) on ACT, out=r3-r4
#              "clamp" -> c2=clamp(u,+-t) on DVE,       out=u-c2
#   sub_engine: engine for the final 2-input subtract
_VARIANTS = {
    "allv": ("vector", "clamp", "vector"),   # all-DVE bitwise-exact chain
    "a":    ("gpsimd", "relu",  "vector"),
    "b":    ("vector", "relu",  "gpsimd"),
    "c":    ("vector", "relu",  "vector"),
    "d":    ("scalar", "relu",  "gpsimd"),
    "e":    ("gpsimd", "clamp", "gpsimd"),
    # "direct"/"directs": out = x - clamp(x, +-lam)  (2 DVE ops; absmax vs
    # reference ~5e-7 instead of bitwise 0).  "direct" puts store-DMAs on the
    # ACT HWDGE ring so they don't share the sync-ring FIFO with loads.
    "direct":  (None, None, None),
    "directs": (None, None, None),
}


def _split_multi_waits(nc):
    """The walrus build here accepts at most ONE sync wait per instruction.
    Peel extra waits onto single-wait NoOps inserted before the instruction on
    the same engine (the serial lowering walrus would otherwise do itself)."""
    for f in nc.m.functions:
        for bb in f.blocks:
            new_insts = []
            for ins in bb.instructions:
                si = ins.sync_info
                if si is not None and si.on_wait and len(si.on_wait) > 1:
                    waits = list(si.on_wait)
                    for w in waits[:-1]:
                        new_insts.append(
                            mybir.InstNoOp(
                                name=nc.get_next_instruction_name(),
                                engine=ins.engine,
                                ins=[],
                                outs=[],
                                sync_info=mybir.SyncInfo(on_wait=[w], on_update=[]),
                            )
                        )
                    si.on_wait = waits[-1:]
                new_insts.append(ins)
            bb.instructions = new_insts


def _build(rho: float, lam: float, nchunk: int = _NCHUNK, variant: str = _VARIANT):
    """Trace the single-core Bass program (rho/lam folded in as immediates)."""
    Alu = mybir.AluOpType
    Act = mybir.ActivationFunctionType
    m_eng, soft_mode, sub_eng = _VARIANTS[variant]
    a = float(1.0 - rho)      # contraction factor
    t = float(rho * lam)      # threshold of the final ISTA step
    lam = float(lam)

    nc = bass.Bass()
    x = nc.declare_dram_parameter("x", [_P, _FD], _f32, isOutput=False)
    y = nc.declare_dram_parameter("y", [_P, _FD], _f32, isOutput=True)

    if soft_mode == "relu" and (_f32, -t) not in nc.const_aps.aps:
        # ACT `activation` requires non-Copy biases as const APs; register -t
        # the same way Bass registers its built-in 0.0/1.0 consts.
        h = nc.alloc_sbuf_tensor("const-f32-bias", [_P, 1], _f32)
        nc.gpsimd.memset(h.ap(), -t)
        nc.const_aps.aps[(_f32, -t)] = h.ap()
        nc.all_engine_barrier()

    direct = variant.startswith("direct")
    store_eng = nc.scalar if variant == "direct" else nc.sync
    W = _FD // nchunk
    with TileContext(nc) as tc:
        with tc.tile_pool(name="io", bufs=3) as pool:
            for c in range(nchunk):
                sl = slice(c * W, (c + 1) * W)
                xin = pool.tile([_P, W], _f32, tag="xin")
                nc.sync.dma_start(out=xin[:], in_=x[:, sl])

                # c1 = clamp(x, +-lam)          (DVE tensor_scalar, 2x mode)
                c1 = pool.tile([_P, W], _f32, tag="c1")
                nc.vector.tensor_scalar(c1[:], xin[:], -lam, lam, Alu.max, Alu.min)

                if direct:
                    out = pool.tile([_P, W], _f32, tag="out")
                    nc.vector.tensor_tensor(out[:], xin[:], c1[:], Alu.subtract)
                    store_eng.dma_start(out=y[:, sl], in_=out[:])
                    continue

                # m = c1 * (-a)
                m = pool.tile([_P, W], _f32, tag="m")
                if m_eng == "scalar":
                    nc.scalar.activation(m[:], c1[:], Act.Copy, bias=0.0, scale=-a)
                else:
                    getattr(nc, m_eng).tensor_scalar_mul(m[:], c1[:], -a)

                # u = m + x
                u = pool.tile([_P, W], _f32, tag="u")
                nc.vector.tensor_tensor(u[:], m[:], xin[:], Alu.add)

                # out = softshrink(u, t)
                out = pool.tile([_P, W], _f32, tag="out")
                if soft_mode == "clamp":
                    c2 = pool.tile([_P, W], _f32, tag="c2")
                    nc.vector.tensor_scalar(c2[:], u[:], -t, t, Alu.max, Alu.min)
                    getattr(nc, sub_eng).tensor_tensor(
                        out[:], u[:], c2[:], Alu.subtract
                    )
                else:
                    r3 = pool.tile([_P, W], _f32, tag="r3")
                    nc.scalar.activation(r3[:], u[:], Act.Relu, bias=-t, scale=1.0)
                    r4 = pool.tile([_P, W], _f32, tag="r4")
                    nc.scalar.activation(r4[:], u[:], Act.Relu, bias=-t, scale=-1.0)
                    getattr(nc, sub_eng).tensor_tensor(
                        out[:], r3[:], r4[:], Alu.subtract
                    )

                nc.sync.dma_start(out=y[:, sl], in_=out[:])
    _split_multi_waits(nc)
    return nc


def _build_raw(rho: float, lam: float, widths):
    """Raw-Bass (no TileContext) pipeline: no prologue/tail all-engine
    barriers.  sync issues loads (SP HWDGE ring), DVE computes
    out = x - clamp(x, +-lam), ACT issues stores (ACT HWDGE ring) and waits
    for their completion.  Each chunk gets dedicated SBUF slots, so the only
    synchronization is load->compute->store along each chunk."""
    Alu = mybir.AluOpType
    lam = float(lam)
    n = len(widths)
    assert sum(widths) == _FD

    nc = bass.Bass()
    x = nc.declare_dram_parameter("x", [_P, _FD], _f32, isOutput=False)
    y = nc.declare_dram_parameter("y", [_P, _FD], _f32, isOutput=True)

    xin = [nc.alloc_sbuf_tensor(f"xin{i}", [_P, w], _f32) for i, w in enumerate(widths)]
    c1 = [nc.alloc_sbuf_tensor(f"c1_{i}", [_P, w], _f32) for i, w in enumerate(widths)]
    out = [nc.alloc_sbuf_tensor(f"out{i}", [_P, w], _f32) for i, w in enumerate(widths)]
    offs = [sum(widths[:i]) for i in range(n)]

    with (
        nc.semaphore("s_in") as s_in,
        nc.semaphore("s_cmp") as s_cmp,
        nc.semaphore("s_out") as s_out,
        nc.Block() as block,
    ):

        @block.sync
        def _(sync):
            for i, w in enumerate(widths):
                sync.dma_start(
                    out=xin[i].ap(), in_=x[:, offs[i] : offs[i] + w]
                ).then_inc(s_in, 16)

        @block.vector
        def _(vector):
            for i, w in enumerate(widths):
                vector.wait_ge(s_in, 16 * (i + 1))
                vector.tensor_scalar(
                    c1[i].ap(), xin[i].ap(), -lam, lam, Alu.max, Alu.min
                )
                vector.tensor_tensor(
                    out[i].ap(), xin[i].ap(), c1[i].ap(), Alu.subtract
                ).then_inc(s_cmp, 1)

        @block.scalar
        def _(scalar):
            for i, w in enumerate(widths):
                scalar.wait_ge(s_cmp, i + 1)
                scalar.dma_start(
                    out=y[:, offs[i] : offs[i] + w], in_=out[i].ap()
                ).then_inc(s_out, 16)
            scalar.wait_ge(s_out, 16 * n)

    _split_multi_waits(nc)
    return nc


def _build_raw2(rho: float, lam: float, widths):
    """Like _build_raw but without nc.Block(), so no block-exit all-engine
    barrier/drain at all.  All instructions live in the main bb, engine-tagged;
    each sequencer executes its own subsequence in order.  The ACT engine's
    final wait on the store semaphore is the only completion guard."""
    Alu = mybir.AluOpType
    lam = float(lam)
    n = len(widths)
    assert sum(widths) == _FD

    nc = bass.Bass()
    x = nc.declare_dram_parameter("x", [_P, _FD], _f32, isOutput=False)
    y = nc.declare_dram_parameter("y", [_P, _FD], _f32, isOutput=True)

    xin = [nc.alloc_sbuf_tensor(f"xin{i}", [_P, w], _f32) for i, w in enumerate(widths)]
    c1 = [nc.alloc_sbuf_tensor(f"c1_{i}", [_P, w], _f32) for i, w in enumerate(widths)]
    out = [nc.alloc_sbuf_tensor(f"out{i}", [_P, w], _f32) for i, w in enumerate(widths)]
    offs = [sum(widths[:i]) for i in range(n)]

    s_in = nc.alloc_semaphore("s_in")
    s_cmp = nc.alloc_semaphore("s_cmp")
    s_out = nc.alloc_semaphore("s_out")

    for i, w in enumerate(widths):
        nc.sync.dma_start(out=xin[i].ap(), in_=x[:, offs[i] : offs[i] + w]).then_inc(
            s_in, 16
        )
    for i, w in enumerate(widths):
        nc.vector.wait_ge(s_in, 16 * (i + 1))
        nc.vector.tensor_scalar(c1[i].ap(), xin[i].ap(), -lam, lam, Alu.max, Alu.min)
        nc.vector.tensor_tensor(
            out[i].ap(), xin[i].ap(), c1[i].ap(), Alu.subtract
        ).then_inc(s_cmp, 1)
    for i, w in enumerate(widths):
        nc.scalar.wait_ge(s_cmp, i + 1)
        nc.scalar.dma_start(
            out=y[:, offs[i] : offs[i] + w], in_=out[i].ap()
        ).then_inc(s_out, 16)
    nc.scalar.wait_ge(s_out, 16 * n)

    _split_multi_waits(nc)
    return nc


_built = {}


def _get_nc(rho: float, lam: float, nchunk: int = _NCHUNK, variant: str = _VARIANT):
    key = (rho, lam, nchunk, variant)
    if key not in _built:
        if variant == "raw":
            w = _FD // nchunk
            _built[key] = _build_raw(rho, lam, [w] * nchunk)
        elif variant == "rawt":
            _built[key] = _build_raw(rho, lam, [2048, 2048, 1536, 512])
        elif variant == "raw2":
            w = _FD // nchunk
            _built[key] = _build_raw2(rho, lam, [w] * nchunk)
        elif variant == "raw2t":
            _built[key] = _build_raw2(rho, lam, [2048, 2048, 1536, 512])
        else:
            _built[key] = _build(rho, lam, nchunk, variant)
    return _built[key]


def _run(x0, rho, lam, nchunk=_NCHUNK, variant=_VARIANT, **spmd_kwargs):
    """Run on 8 cores; returns (full_output, BassKernelResults)."""
    x0 = np.ascontiguousarray(np.asarray(x0, dtype=np.float32))
    assert x0.shape == (_B, _C, _H, _W), x0.shape
    rho_f = float(np.asarray(rho))
    lam_f = float(np.asarray(lam))

    nc = _get_nc(rho_f, lam_f, nchunk, variant)
    xs = x0.reshape(_B, _P, _FD)
    in_maps = [{"x": xs[i]} for i in range(_NCORES)]
    res = run_bass_kernel_spmd(nc, in_maps, list(range(_NCORES)), **spmd_kwargs)
    out = np.stack(
        [res.results[i]["y"].reshape(_C, _H, _W) for i in range(_NCORES)], axis=0
    )
    return np.ascontiguousarray(out, dtype=np.float32), res


def kernel(x0, rho, lam):
    out, _ = _run(x0, rho, lam)
    return out


# revision 12
# speedup vs baseline: 1.0195x; 1.0195x over previous
"""Trainium2 Bass kernel for nn_DEQSolver_2894807957574.

Math: the reference runs 40 Anderson-accelerated fixed-point iterations of the
ISTA map  f(z) = softshrink((1-rho)*z + rho*x0, rho*lam)  and then applies one
more ISTA step.  The map is a contraction with factor |1-rho| (= 0.1 here), so
in fp32 the iterate fully converges to the unique fixed point
z* = softshrink(x0, lam), and the final ISTA step maps the fixed point to
itself.  The returned value therefore has the closed form

    c1 = clamp(x0, +-lam)
    u  = x0 + (-(1-rho))*c1          # = (1-rho)*z* + rho*x0
    out = u - clamp(u, +-rho*lam)    # = softshrink(u, rho*lam)

computed with exactly this fp32 op order; that was verified BITWISE identical
(absmax diff 0.0) to the full jax reference on the target inputs.

Sharding: pure data parallel - batch dim 8, one sample per NeuronCore.  Each
core streams its 3 MB sample HBM->SBUF in chunks, applies the elementwise ops
(optionally split across DVE / ACT / GPSIMD so the ~358 GB/s HBM path stays
the bottleneck), and streams the 3 MB result back.
"""

import numpy as np

import concourse.bass as bass
import concourse.mybir as mybir
from concourse.bass_utils import run_bass_kernel_spmd
from concourse.tile import TileContext

_B, _C, _H, _W = 8, 3, 512, 512
_P = 128                      # SBUF partitions
_FD = (_C * _H * _W) // _P    # 6144 free-dim elements per partition
_NCORES = 8
_NCHUNK = 4                   # chunks along the free dim (768 KB per DMA)
_VARIANT = "b"

_f32 = mybir.dt.float32

# variant -> (m_engine, soft_mode, sub_engine)
#   m_engine: engine computing m = c1 * (-(1-rho))
#   soft_mode: "relu"  -> r3=relu(u-t), r4=relu(-u-t) on ACT, out=r3-r4
#              "clamp" -> c2=clamp(u,+-t) on DVE,       out=u-c2
#   sub_engine: engine for the final 2-input subtract
_VARIANTS = {
    "allv": ("vector", "clamp", "vector"),   # all-DVE bitwise-exact chain
    "a":    ("gpsimd", "relu",  "vector"),
    "b":    ("vector", "relu",  "gpsimd"),
    "c":    ("vector", "relu",  "vector"),
    "d":    ("scalar", "relu",  "gpsimd"),
    "e":    ("gpsimd", "clamp", "gpsimd"),
    # "direct"/"directs": out = x - clamp(x, +-lam)  (2 DVE ops; absmax vs
    # reference ~5e-7 instead of bitwise 0).  "direct" puts store-DMAs on the
    # ACT HWDGE ring so they don't share the sync-ring FIFO with loads.
    "direct":  (None, None, None),
    "directs": (None, None, None),
}


def _split_multi_waits(nc):
    """The walrus build here accepts at most ONE sync wait per instruction.
    Peel extra waits onto single-wait NoOps inserted before the instruction on
    the same engine (the serial lowering walrus would otherwise do itself)."""
    for f in nc.m.functions:
        for bb in f.blocks:
            new_insts = []
            for ins in bb.instructions:
                si = ins.sync_info
                if si is not None and si.on_wait and len(si.on_wait) > 1:
                    waits = list(si.on_wait)
                    for w in waits[:-1]:
                        new_insts.append(
                            mybir.InstNoOp(
                                name=nc.get_next_instruction_name(),
                                engine=ins.engine,
                                ins=[],
                                outs=[],
                                sync_info=mybir.SyncInfo(on_wait=[w], on_update=[]),
                            )
                        )
                    si.on_wait = waits[-1:]
                new_insts.append(ins)
            bb.instructions = new_insts


def _build(rho: float, lam: float, nchunk: int = _NCHUNK, variant: str = _VARIANT):
    """Trace the single-core Bass program (rho/lam folded in as immediates)."""
    Alu = mybir.AluOpType
    Act = mybir.ActivationFunctionType
    m_eng, soft_mode, sub_eng = _VARIANTS[variant]
    a = float(1.0 - rho)      # contraction factor
    t = float(rho * lam)      # threshold of the final ISTA step
    lam = float(lam)

    nc = bass.Bass()
    x = nc.declare_dram_parameter("x", [_P, _FD], _f32, isOutput=False)
    y = nc.declare_dram_parameter("y", [_P, _FD], _f32, isOutput=True)

    if soft_mode == "relu" and (_f32, -t) not in nc.const_aps.aps:
        # ACT `activation` requires non-Copy biases as const APs; register -t
        # the same way Bass registers its built-in 0.0/1.0 consts.
        h = nc.alloc_sbuf_tensor("const-f32-bias", [_P, 1], _f32)
        nc.gpsimd.memset(h.ap(), -t)
        nc.const_aps.aps[(_f32, -t)] = h.ap()
        nc.all_engine_barrier()

    direct = variant.startswith("direct")
    store_eng = nc.scalar if variant == "direct" else nc.sync
    W = _FD // nchunk
    with TileContext(nc) as tc:
        with tc.tile_pool(name="io", bufs=3) as pool:
            for c in range(nchunk):
                sl = slice(c * W, (c + 1) * W)
                xin = pool.tile([_P, W], _f32, tag="xin")
                nc.sync.dma_start(out=xin[:], in_=x[:, sl])

                # c1 = clamp(x, +-lam)          (DVE tensor_scalar, 2x mode)
                c1 = pool.tile([_P, W], _f32, tag="c1")
                nc.vector.tensor_scalar(c1[:], xin[:], -lam, lam, Alu.max, Alu.min)

                if direct:
                    out = pool.tile([_P, W], _f32, tag="out")
                    nc.vector.tensor_tensor(out[:], xin[:], c1[:], Alu.subtract)
                    store_eng.dma_start(out=y[:, sl], in_=out[:])
                    continue

                # m = c1 * (-a)
                m = pool.tile([_P, W], _f32, tag="m")
                if m_eng == "scalar":
                    nc.scalar.activation(m[:], c1[:], Act.Copy, bias=0.0, scale=-a)
                else:
                    getattr(nc, m_eng).tensor_scalar_mul(m[:], c1[:], -a)

                # u = m + x
                u = pool.tile([_P, W], _f32, tag="u")
                nc.vector.tensor_tensor(u[:], m[:], xin[:], Alu.add)

                # out = softshrink(u, t)
                out = pool.tile([_P, W], _f32, tag="out")
                if soft_mode == "clamp":
                    c2 = pool.tile([_P, W], _f32, tag="c2")
                    nc.vector.tensor_scalar(c2[:], u[:], -t, t, Alu.max, Alu.min)
                    getattr(nc, sub_eng).tensor_tensor(
                        out[:], u[:], c2[:], Alu.subtract
                    )
                else:
                    r3 = pool.tile([_P, W], _f32, tag="r3")
                    nc.scalar.activation(r3[:], u[:], Act.Relu, bias=-t, scale=1.0)
                    r4 = pool.tile([_P, W], _f32, tag="r4")
                    nc.scalar.activation(r4[:], u[:], Act.Relu, bias=-t, scale=-1.0)
                    getattr(nc, sub_eng).tensor_tensor(
                        out[:], r3[:], r4[:], Alu.subtract
                    )

                nc.sync.dma_start(out=y[:, sl], in_=out[:])
    _split_multi_waits(nc)
    return nc


def _build_raw(rho: float, lam: float, widths):
    """Raw-Bass (no TileContext) pipeline: no prologue/tail all-engine
    barriers.  sync issues loads (SP HWDGE ring), DVE computes
    out = x - clamp(x, +-lam), ACT issues stores (ACT HWDGE ring) and waits
    for their completion.  Each chunk gets dedicated SBUF slots, so the only
    synchronization is load->compute->store along each chunk."""
    Alu = mybir.AluOpType
    lam = float(lam)
    n = len(widths)
    assert sum(widths) == _FD

    nc = bass.Bass()
    x = nc.declare_dram_parameter("x", [_P, _FD], _f32, isOutput=False)
    y = nc.declare_dram_parameter("y", [_P, _FD], _f32, isOutput=True)

    xin = [nc.alloc_sbuf_tensor(f"xin{i}", [_P, w], _f32) for i, w in enumerate(widths)]
    c1 = [nc.alloc_sbuf_tensor(f"c1_{i}", [_P, w], _f32) for i, w in enumerate(widths)]
    out = [nc.alloc_sbuf_tensor(f"out{i}", [_P, w], _f32) for i, w in enumerate(widths)]
    offs = [sum(widths[:i]) for i in range(n)]

    with (
        nc.semaphore("s_in") as s_in,
        nc.semaphore("s_cmp") as s_cmp,
        nc.semaphore("s_out") as s_out,
        nc.Block() as block,
    ):

        @block.sync
        def _(sync):
            for i, w in enumerate(widths):
                sync.dma_start(
                    out=xin[i].ap(), in_=x[:, offs[i] : offs[i] + w]
                ).then_inc(s_in, 16)

        @block.vector
        def _(vector):
            for i, w in enumerate(widths):
                vector.wait_ge(s_in, 16 * (i + 1))
                vector.tensor_scalar(
                    c1[i].ap(), xin[i].ap(), -lam, lam, Alu.max, Alu.min
                )
                vector.tensor_tensor(
                    out[i].ap(), xin[i].ap(), c1[i].ap(), Alu.subtract
                ).then_inc(s_cmp, 1)

        @block.scalar
        def _(scalar):
            for i, w in enumerate(widths):
                scalar.wait_ge(s_cmp, i + 1)
                scalar.dma_start(
                    out=y[:, offs[i] : offs[i] + w], in_=out[i].ap()
                ).then_inc(s_out, 16)
            scalar.wait_ge(s_out, 16 * n)

    _split_multi_waits(nc)
    return nc


def _build_raw2(rho: float, lam: float, widths):
    """Like _build_raw but without nc.Block(), so no block-exit all-engine
    barrier/drain at all.  All instructions live in the main bb, engine-tagged;
    each sequencer executes its own subsequence in order.  The ACT engine's
    final wait on the store semaphore is the only completion guard."""
    Alu = mybir.AluOpType
    lam = float(lam)
    n = len(widths)
    assert sum(widths) == _FD

    nc = bass.Bass()
    x = nc.declare_dram_parameter("x", [_P, _FD], _f32, isOutput=False)
    y = nc.declare_dram_parameter("y", [_P, _FD], _f32, isOutput=True)

    xin = [nc.alloc_sbuf_tensor(f"xin{i}", [_P, w], _f32) for i, w in enumerate(widths)]
    c1 = [nc.alloc_sbuf_tensor(f"c1_{i}", [_P, w], _f32) for i, w in enumerate(widths)]
    out = [nc.alloc_sbuf_tensor(f"out{i}", [_P, w], _f32) for i, w in enumerate(widths)]
    offs = [sum(widths[:i]) for i in range(n)]

    s_in = nc.alloc_semaphore("s_in")
    s_cmp = nc.alloc_semaphore("s_cmp")
    s_out = nc.alloc_semaphore("s_out")

    for i, w in enumerate(widths):
        nc.sync.dma_start(out=xin[i].ap(), in_=x[:, offs[i] : offs[i] + w]).then_inc(
            s_in, 16
        )
    for i, w in enumerate(widths):
        nc.vector.wait_ge(s_in, 16 * (i + 1))
        nc.vector.tensor_scalar(c1[i].ap(), xin[i].ap(), -lam, lam, Alu.max, Alu.min)
        nc.vector.tensor_tensor(
            out[i].ap(), xin[i].ap(), c1[i].ap(), Alu.subtract
        ).then_inc(s_cmp, 1)
    for i, w in enumerate(widths):
        nc.scalar.wait_ge(s_cmp, i + 1)
        nc.scalar.dma_start(
            out=y[:, offs[i] : offs[i] + w], in_=out[i].ap()
        ).then_inc(s_out, 16)
    nc.scalar.wait_ge(s_out, 16 * n)

    _split_multi_waits(nc)
    return nc


_built = {}


def _get_nc(rho: float, lam: float, nchunk: int = _NCHUNK, variant: str = _VARIANT):
    key = (rho, lam, nchunk, variant)
    if key not in _built:
        if variant == "raw":
            w = _FD // nchunk
            _built[key] = _build_raw(rho, lam, [w] * nchunk)
        elif variant == "rawt":
            _built[key] = _build_raw(rho, lam, [2048, 2048, 1536, 512])
        elif variant == "raw2":
            w = _FD // nchunk
            _built[key] = _build_raw2(rho, lam, [w] * nchunk)
        elif variant == "raw2t":
            _built[key] = _build_raw2(rho, lam, [2048, 2048, 1536, 512])
        elif variant == "raw2h":
            _built[key] = _build_raw2(rho, lam, [512, 1536, 2048, 1536, 512])
        else:
            _built[key] = _build(rho, lam, nchunk, variant)
    return _built[key]


def _run(x0, rho, lam, nchunk=_NCHUNK, variant=_VARIANT, **spmd_kwargs):
    """Run on 8 cores; returns (full_output, BassKernelResults)."""
    x0 = np.ascontiguousarray(np.asarray(x0, dtype=np.float32))
    assert x0.shape == (_B, _C, _H, _W), x0.shape
    rho_f = float(np.asarray(rho))
    lam_f = float(np.asarray(lam))

    nc = _get_nc(rho_f, lam_f, nchunk, variant)
    xs = x0.reshape(_B, _P, _FD)
    in_maps = [{"x": xs[i]} for i in range(_NCORES)]
    res = run_bass_kernel_spmd(nc, in_maps, list(range(_NCORES)), **spmd_kwargs)
    out = np.stack(
        [res.results[i]["y"].reshape(_C, _H, _W) for i in range(_NCORES)], axis=0
    )
    return np.ascontiguousarray(out, dtype=np.float32), res


def kernel(x0, rho, lam):
    out, _ = _run(x0, rho, lam)
    return out
